# revision 1
# baseline (speedup 1.0000x reference)
"""Causal flash attention (B=2, H=16, S=2048, D=64, fp32) on 8 TRN2 NeuronCores.

Strategy: shard batch*heads (32) across 8 cores -> 4 heads/core. Per head,
compute transposed scores S^T[k, q] = K Q^T via PE (fp16 inputs, fp32 PSUM
accumulate), exp on ACT (softmax scale folded into the activation input
scale, output rounded to fp16), causal mask applied post-exp as a
multiplicative 0/1 fp16 mask on the two diagonal tiles (DVE), then PV via
PE with a ones column appended to V so the softmax denominator falls out of
the same matmul. The output leaves the device transposed ([d+1, q] per
head, fp16); the host divides by the denominator row and transposes back.

Two heads are packed into the 128 SBUF partitions (d=64 each) so QK matmuls
for a head pair run concurrently on disjoint PE row groups. The ACT engine
(exp at 1 elem/lane/cycle) is the bottleneck, so scores are exponentiated
in [128, 1024] fp32 groups (2 PSUM banks, double-buffered); ACT's marginal
rate is ~1.17 cyc/elem so larger groups buy nothing.

PSUM bank rules (hardware-faulting otherwise): the two concurrently
executing row-group matmuls of a QK pair must write DIFFERENT PSUM banks,
and no matmul output may cross a bank boundary. Groups use the
head-major layout A0 A1 | B0 B1 so pair members land in distinct banks.
Also, start=True clears has_written for the whole bank, so the per-head PV
accumulators live in separate banks with exactly one start each.
"""

import numpy as np

B, H, S, D = 2, 16, 2048, 64
BH = B * H
NCORES = 8
HPC = BH // NCORES  # heads per core
SCALE = 0.125
W = 256             # q-block width
TK = 128            # k-tile height
NKT = S // TK       # 16 k-tiles
NQB = S // W        # 8 q-blocks
GW = 512            # max score-group width per head (2 PSUM banks total)

# Bank-safe column offsets (fp32 cols): banks are 512 cols (2KB); the two
# concurrently executing row-group matmuls of a (A_j, B_j) pair must land in
# different banks and no matmul output may cross a bank boundary.
_OFFS_A = [0, 256]
_OFFS_B = [512, 768]

_CACHE = {}


def _pack_groups(nkt):
    """nkt full k-tiles in pairs."""
    groups = []
    kt = 0
    while kt < nkt:
        groups.append([(kt, W), (kt + 1, W)])
        kt += 2
    return groups


def _build_nc():
    import concourse.bass as bass  # noqa: F401
    import concourse.mybir as mybir
    import concourse.tile as tile
    from concourse import bacc

    f32 = mybir.dt.float32
    f16 = mybir.dt.float16
    EXP = mybir.ActivationFunctionType.Exp

    nc = bacc.Bacc("TRN2", target_bir_lowering=False, debug=False, num_devices=NCORES)

    # Host-swizzled layouts so every DMA reads multi-KB contiguous rows.
    kt_d = nc.dram_tensor("ktp", [128, 2, S], f16, kind="ExternalInput").ap()
    qt_d = nc.dram_tensor("qtp", [128, 2, S], f16, kind="ExternalInput").ap()
    v_d = nc.dram_tensor("vxp", [128, HPC, NKT, D + 1], f16,
                         kind="ExternalInput").ap()
    o_d = nc.dram_tensor("outT", [HPC, D + 1, S], f16, kind="ExternalOutput").ap()

    with tile.TileContext(nc) as tc:
        const_pool = tc.alloc_tile_pool(name="const", bufs=1)
        kq_pool = tc.alloc_tile_pool(name="kq", bufs=1)
        vx_pool = tc.alloc_tile_pool(name="vx", bufs=1)
        p_pool = tc.alloc_tile_pool(name="p", bufs=6)
        o_pool = tc.alloc_tile_pool(name="o", bufs=8)
        ps_pool = tc.alloc_tile_pool(name="ps", bufs=3, space="PSUM")
        pv_pool = tc.alloc_tile_pool(name="pv", bufs=2, space="PSUM")

        ktpp = [kq_pool.tile([128, S], f16, name=f"ktp{r}", tag=f"ktp{r}")
                for r in range(2)]
        qtpp = [kq_pool.tile([128, S], f16, name=f"qtp{r}", tag=f"qtp{r}")
                for r in range(2)]
        vxtp = [vx_pool.tile([128, 2, NKT, D + 1], f16, name=f"vxt{r}",
                             tag=f"vxt{r}") for r in range(2)]

        # Input loads, first-needed pieces first (qb descends, k-tiles
        # ascend within a q-block); per-pair tiles so pr0 compute only
        # depends on pr0 chunks. v rides the gpsimd SWDGE queue in parallel.
        # k on the sync HWDGE ring, q on the scalar HWDGE ring (parallel
        # transfers; the scalar engine is idle until its first activate),
        # v on the gpsimd SWDGE ring.
        nc.sync.dma_start(ktpp[0][:, 0:256], kt_d[:, 0, 0:256])
        nc.scalar.dma_start(qtpp[0][:, 1792:S], qt_d[:, 0, 1792:S])
        nc.gpsimd.dma_start(vxtp[0][:, :, 0:2], v_d[:, 0:2, 0:2])
        nc.sync.dma_start(ktpp[0][:, 256:768], kt_d[:, 0, 256:768])
        nc.gpsimd.dma_start(vxtp[0][:, :, 2:6], v_d[:, 0:2, 2:6])
        nc.scalar.dma_start(qtpp[0][:, 1536:1792], qt_d[:, 0, 1536:1792])
        nc.sync.dma_start(ktpp[0][:, 768:1280], kt_d[:, 0, 768:1280])
        nc.gpsimd.dma_start(vxtp[0][:, :, 6:10], v_d[:, 0:2, 6:10])
        nc.sync.dma_start(ktpp[0][:, 1280:S], kt_d[:, 0, 1280:S])
        nc.scalar.dma_start(qtpp[0][:, 768:1536], qt_d[:, 0, 768:1536])
        nc.gpsimd.dma_start(vxtp[0][:, :, 10:NKT], v_d[:, 0:2, 10:NKT])
        nc.scalar.dma_start(qtpp[0][:, 0:768], qt_d[:, 0, 0:768])
        nc.sync.dma_start(ktpp[1][:], kt_d[:, 1, :])
        nc.scalar.dma_start(qtpp[1][:], qt_d[:, 1, :])
        nc.gpsimd.dma_start(vxtp[1][:], v_d[:, 2:4])

        # Multiplicative causal masks for the two diagonal k-tiles of each
        # q-block. maskA[x, y] = 1 if y >= x ; maskB: 1 if y >= x + 128.
        maskA = const_pool.tile([128, W], f16, name="maskA")
        maskB = const_pool.tile([128, W], f16, name="maskB")
        for m, base in ((maskA, 0), (maskB, -128)):
            nc.gpsimd.memset(m[:], 1.0)
            nc.gpsimd.affine_select(
                out=m[:], in_=m[:],
                compare_op=mybir.AluOpType.is_ge,
                fill=0.0, base=base,
                pattern=[[1, W]], channel_multiplier=-1,
            )

        # Main pipeline over a flat unit list (pr, qb, group) so the pr
        # boundary pipelines like any group boundary: pr1's first QK is
        # emitted before pr0's final PV flush. QK runs a group ahead of the
        # exp that consumes it; PV matmuls lag one group behind the exp.
        units = []
        for pr in range(2):
            for qb in reversed(range(NQB)):
                nkt = 2 * qb + 2
                groups = _pack_groups(nkt)
                for gi, group in enumerate(groups):
                    units.append((pr, qb, nkt, group, gi == 0,
                                  gi == len(groups) - 1))

        state = {"pending": None}

        def flush_pending():
            pending = state["pending"]
            if pending is None:
                return
            pr, nkt, group, p, pvA, pvB, out_qb = pending
            vxt = vxtp[pr]
            for j, (kt, w) in enumerate(group):
                start = kt == 0
                stop = kt == nkt - 1
                nc.tensor.matmul(
                    pvA[:, W - w:W], vxt[:, 0, kt, :],
                    p[:, _OFFS_A[j]:_OFFS_A[j] + w],
                    start=start, stop=stop, skip_group_check=True,
                )
                nc.tensor.matmul(
                    pvB[:, W - w:W], vxt[:, 1, kt, :],
                    p[:, _OFFS_B[j]:_OFFS_B[j] + w],
                    start=start, stop=stop, skip_group_check=True,
                )
            if out_qb is not None:  # last group of the q-block: write out
                hA, hB = 2 * pr, 2 * pr + 1
                oA = o_pool.tile([D + 1, W], f16, tag="o", name="oA")
                oB = o_pool.tile([D + 1, W], f16, tag="o", name="oB")
                nc.vector.tensor_copy(oA[:], pvA[:])
                nc.vector.tensor_copy(oB[:], pvB[:])
                nc.sync.dma_start(
                    o_d[hA, :, out_qb * W:(out_qb + 1) * W], oA[:])
                # tail q-blocks: second head's store on the scalar HWDGE
                # ring (idle after its last activate) to shorten the final
                # dispatch+receipt chain.
                dma_b = nc.scalar if pr == 1 and out_qb <= 1 else nc.sync
                dma_b.dma_start(
                    o_d[hB, :, out_qb * W:(out_qb + 1) * W], oB[:])
            state["pending"] = None

        pvA = pvB = None
        for pr, qb, nkt, group, first_g, last_g in units:
            if first_g:
                pvA = pv_pool.tile([D + 1, W], f32, tag="pv", name="pvA")
                pvB = pv_pool.tile([D + 1, W], f32, tag="pv", name="pvB")
            ktp = ktpp[pr]
            qtp = qtpp[pr]
            span = 1024
            sG = ps_pool.tile([128, 2 * GW], f32, tag="sG", name="sG")
            for j, (kt, w) in enumerate(group):
                q0 = qb * W + (W - w)
                nc.tensor.matmul(
                    sG[:, _OFFS_A[j]:_OFFS_A[j] + w],
                    ktp[0:64, kt * TK:(kt + 1) * TK],
                    qtp[0:64, q0:q0 + w],
                    start=True, stop=True,
                )
                nc.tensor.matmul(
                    sG[:, _OFFS_B[j]:_OFFS_B[j] + w],
                    ktp[64:128, kt * TK:(kt + 1) * TK],
                    qtp[64:128, q0:q0 + w],
                    start=True, stop=True,
                )
            p = p_pool.tile([128, 2 * GW], f16, tag="p", name="p")
            nc.scalar.activation(p[:, :span], sG[:, :span], EXP, scale=SCALE)
            # multiplicative causal mask on the diagonal tiles
            for j, (kt, w) in enumerate(group):
                mask = (maskA if kt == nkt - 2
                        else maskB if kt == nkt - 1 else None)
                if mask is None:
                    continue
                for off in (_OFFS_A[j], _OFFS_B[j]):
                    nc.vector.tensor_mul(
                        p[:, off:off + w], p[:, off:off + w], mask[:],
                    )
            flush_pending()
            state["pending"] = (pr, nkt, group, p, pvA, pvB,
                                qb if last_g else None)
        flush_pending()

        pv_pool.release()
        ps_pool.release()
        o_pool.release()
        p_pool.release()
        vx_pool.release()
        kq_pool.release()
        const_pool.release()

    nc.compile()
    return nc


def _get_nc():
    if "nc" not in _CACHE:
        _CACHE["nc"] = _build_nc()
    return _CACHE["nc"]


def _prep_inputs(q, k, v):
    qf = np.ascontiguousarray(np.asarray(q, dtype=np.float32)).reshape(BH, S, D)
    kf = np.ascontiguousarray(np.asarray(k, dtype=np.float32)).reshape(BH, S, D)
    vf = np.ascontiguousarray(np.asarray(v, dtype=np.float32)).reshape(BH, S, D)
    vx = np.empty((BH, S, D + 1), np.float16)
    vx[:, :, :D] = vf
    vx[:, :, D] = 1.0
    qt = qf.transpose(0, 2, 1).astype(np.float16)  # [BH, D, S]
    kt = kf.transpose(0, 2, 1).astype(np.float16)
    in_maps = []
    for c in range(NCORES):
        sl = slice(HPC * c, HPC * (c + 1))
        # [128, 2, S]: partition = (head-in-pair, d), middle = pair index
        ktp = kt[sl].reshape(2, 128, S).transpose(1, 0, 2)
        qtp = qt[sl].reshape(2, 128, S).transpose(1, 0, 2)
        # [128, HPC, NKT, D+1]: partition = kv offset within k-tile
        vxp = vx[sl].reshape(HPC, NKT, TK, D + 1).transpose(2, 0, 1, 3)
        in_maps.append({
            "ktp": np.ascontiguousarray(ktp),
            "qtp": np.ascontiguousarray(qtp),
            "vxp": np.ascontiguousarray(vxp),
        })
    return in_maps


def _postprocess(results):
    out = np.empty((B, H, S, D), np.float32)
    for c in range(NCORES):
        ot = results[c]["outT"].astype(np.float32)  # [HPC, D+1, S]
        o = (ot[:, :D, :] / ot[:, D:D + 1, :]).transpose(0, 2, 1)  # [HPC, S, D]
        for i in range(HPC):
            bh = HPC * c + i
            out[bh // H, bh % H] = o[i]
    return out


def run(q, k, v, trace=False):
    from concourse.bass_utils import run_bass_kernel_spmd

    nc = _get_nc()
    in_maps = _prep_inputs(q, k, v)
    res = run_bass_kernel_spmd(
        nc, in_maps, core_ids=list(range(NCORES)), trace=trace
    )
    return _postprocess(res.results), res


def kernel(q, k, v):
    out, _ = run(q, k, v, trace=False)
    return out

